# revision 6
# baseline (speedup 1.0000x reference)
"""Trainium2 Bass kernel for nn_FeatureEmbedding (4-layer 3x3 conv CNN
with LeakyReLU + sinusoidal positional-encoding add).

Strategy
--------
Data-parallel over the batch dim: 32 batches x 12 frames = 384 images;
each of the 8 NeuronCores processes 48 images (4 batches).

Per image, the whole layer chain runs out of SBUF:
  - Input is DMAed as 9 shifted copies of the zero-padded image into a
    [45, 66*66] f32r buffer, folding the (kh, kw, cin) taps of layer 1
    into the matmul contraction dim (K=45, one matmul per output tile).
  - Layers 2-4 use shift-GEMM: activations live in zero-padded
    [C, 66*66] f32r buffers; for each 512-pixel output tile the 9 taps
    are accumulated into one PSUM bank with strided rhs access patterns
    (out = sum_taps w_tap.T @ shifted(x)).
  - PSUM is drained by ScalarE: out = Lrelu(psum + bias) written
    (strided) into the next layer's padded f32r buffer.
  - Layer 4 output goes to a flat [128, 4096] f32 buffer; VectorE adds
    the per-(t, channel) positional-encoding scalar; DMA to DRAM.

Matmuls run in float32r (TF32-like, ~1e-4 relative accuracy, full PE
rate at this moving size); accumulation is fp32 in PSUM. Two buffer
sets alternate between images so DMA/PE/ACT pipeline across images.
The conv weights are tiny and pre-marshaled on the host into the
[K, M] stationary layouts the PE wants; the PE table (pure function of
shapes) is precomputed on the host and passed in as a constant.
"""

import numpy as np

import concourse.bass as bass
import concourse.bacc as bacc
import concourse.mybir as mybir
import concourse.tile as tile
from concourse.bass_utils import run_bass_kernel_spmd

F32 = mybir.dt.float32
F32R = mybir.dt.float32r
AF = mybir.ActivationFunctionType

N_CORES = 8
B, T, CIN, H, W = 32, 12, 5, 64, 64
CH = [64, 128, 128, 128]
PITCH = W + 2          # 66
PAD = PITCH * PITCH    # 4356
NPIX = H * W           # 4096
NTILE = 8              # 512-pixel output tiles per image
ROWS_PER_TILE = H // NTILE  # 8
TILEPIX = ROWS_PER_TILE * W  # 512
ALPHA = 0.01           # LeakyReLU negative slope


def _build(nimg: int):
    """Build the per-core Bass program (SPMD: same program on all cores)."""
    nc = bacc.Bacc("TRN2", target_bir_lowering=False, debug=False)

    # x and weights are pre-rounded to f32r on the host (see _round_f32r),
    # so all DMAs are plain HWDGE copies, no on-chip casting.
    xin = nc.dram_tensor("xin", [nimg, CIN, H, W], F32R, kind="ExternalInput")
    w1d = nc.dram_tensor("w1", [CIN * 9, CH[0]], F32R, kind="ExternalInput")
    # layer-2 weights split into kw∈{0,1} pairs (K=128) and kw=2 singles
    w2pd = nc.dram_tensor("w2p", [2 * CH[0], 3 * CH[1]], F32R,
                          kind="ExternalInput")
    w2sd = nc.dram_tensor("w2s", [CH[0], 3 * CH[1]], F32R,
                          kind="ExternalInput")
    w3d = nc.dram_tensor("w3", [CH[1], 9 * CH[2]], F32R, kind="ExternalInput")
    w4d = nc.dram_tensor("w4", [CH[2], 9 * CH[3]], F32R, kind="ExternalInput")
    b1d = nc.dram_tensor("b1", [CH[0], 1], F32, kind="ExternalInput")
    b2d = nc.dram_tensor("b2", [CH[1], 1], F32, kind="ExternalInput")
    b3d = nc.dram_tensor("b3", [CH[2], 1], F32, kind="ExternalInput")
    b4d = nc.dram_tensor("b4", [CH[3], 1], F32, kind="ExternalInput")
    ped = nc.dram_tensor("pe", [CH[3], T], F32, kind="ExternalInput")
    outd = nc.dram_tensor("out", [nimg, CH[3], NPIX], F32, kind="ExternalOutput")

    with tile.TileContext(nc) as tc:
        with (
            tc.tile_pool(name="wpool", bufs=1) as wp,
            tc.tile_pool(name="bpool", bufs=1) as bp,
            tc.tile_pool(name="psum", bufs=8, space="PSUM") as pp,
        ):
            # --- constants ---
            w1s = wp.tile([CIN * 9, CH[0]], F32R)
            nc.sync.dma_start(out=w1s, in_=w1d[:, :])
            w2ps = wp.tile([2 * CH[0], 3 * CH[1]], F32R)
            nc.sync.dma_start(out=w2ps, in_=w2pd[:, :])
            w2ss = wp.tile([CH[0], 3 * CH[1]], F32R)
            nc.sync.dma_start(out=w2ss, in_=w2sd[:, :])
            w3s = wp.tile([CH[1], 9 * CH[2]], F32R)
            nc.sync.dma_start(out=w3s, in_=w3d[:, :])
            w4s = wp.tile([CH[2], 9 * CH[3]], F32R)
            nc.sync.dma_start(out=w4s, in_=w4d[:, :])
            b1s = wp.tile([CH[0], 1], F32)
            nc.sync.dma_start(out=b1s, in_=b1d[:, :])
            b2s = wp.tile([CH[1], 1], F32)
            nc.sync.dma_start(out=b2s, in_=b2d[:, :])
            b3s = wp.tile([CH[2], 1], F32)
            nc.sync.dma_start(out=b3s, in_=b3d[:, :])
            b4s = wp.tile([CH[3], 1], F32)
            nc.sync.dma_start(out=b4s, in_=b4d[:, :])
            pes = wp.tile([CH[3], T], F32)
            nc.sync.dma_start(out=pes, in_=ped[:, :])

            # --- persistent padded activation buffers, double-buffered ---
            sets = []
            for s in range(2):
                x9 = bp.tile([CIN * 9, PAD], F32R, name=f"x9_{s}")
                # h1 holds two copies: partitions 0-63 at shift 0 (copy A),
                # partitions 64-127 shifted one column left (copy B), so
                # layer 2 can pair kw∈{0,1} taps into K=128 matmuls.
                h1 = bp.tile([2 * CH[0], PAD], F32R, name=f"h1_{s}")
                h2 = bp.tile([CH[1], PAD], F32R, name=f"h2_{s}")
                h3 = bp.tile([CH[2], PAD], F32R, name=f"h3_{s}")
                h4 = bp.tile([CH[3], NPIX], F32, name=f"h4_{s}")
                # zero once: interiors are rewritten per image, halos stay 0
                for buf in (x9, h1, h2, h3):
                    nc.vector.memset(buf.bitcast(F32), 0.0)
                sets.append((x9, h1, h2, h3, h4))

            taps = [(kh, kw) for kh in range(3) for kw in range(3)]

            for img in range(nimg):
                x9, h1, h2, h3, h4 = sets[img % 2]
                t = img % T
                x9v = x9.rearrange("p (r c) -> p r c", c=PITCH)
                h1v = h1.rearrange("p (r c) -> p r c", c=PITCH)
                h2v = h2.rearrange("p (r c) -> p r c", c=PITCH)
                h3v = h3.rearrange("p (r c) -> p r c", c=PITCH)

                # 1) input: 9 shifted (clipped) copies of the padded image
                for kh, kw in taps:
                    tap = kh * 3 + kw
                    ih0 = max(0, kh - 1)
                    iw0 = max(0, kw - 1)
                    r0 = 1 + ih0 - kh
                    c0 = 1 + iw0 - kw
                    dst = x9v[tap * CIN:(tap + 1) * CIN,
                              r0:r0 + (H - ih0), c0:c0 + (W - iw0)]
                    src = xin[img, :, ih0:, iw0:]
                    nc.sync.dma_start(out=dst, in_=src)

                # 2) layer 1: K folds (kh, kw, cin); one matmul per tile.
                # ACT writes copy A; a SBUF->SBUF DMA mirrors it into copy B
                # (partitions 64-127) shifted one column left for L2 pairing.
                for j in range(NTILE):
                    r0 = j * ROWS_PER_TILE
                    ps = pp.tile([CH[0], TILEPIX], F32, name=f"ps1_{img}_{j}",
                                 tag="ps")
                    nc.tensor.matmul(
                        ps, w1s, x9v[:, r0:r0 + ROWS_PER_TILE, 0:W],
                        start=True, stop=True)
                    nc.scalar.activation(
                        h1v[0:CH[0], 1 + r0:1 + r0 + ROWS_PER_TILE, 1:1 + W],
                        ps, AF.Lrelu, bias=b1s[:, 0:1], scale=1.0, alpha=ALPHA)
                    nc.sync.dma_start(
                        out=h1v[CH[0]:2 * CH[0],
                                1 + r0:1 + r0 + ROWS_PER_TILE, 0:W],
                        in_=h1v[0:CH[0],
                                1 + r0:1 + r0 + ROWS_PER_TILE, 1:1 + W])

                # 3) layer 2: 3 paired matmuls (kw 0+1, K=128) + 3 single
                # matmuls (kw=2, K=64), all accumulating into one bank
                for j in range(NTILE):
                    r0 = j * ROWS_PER_TILE
                    ps = pp.tile([CH[1], TILEPIX], F32, name=f"ps2_{img}_{j}",
                                 tag="ps")
                    for kh in range(3):
                        nc.tensor.matmul(
                            ps, w2ps[:, kh * CH[1]:(kh + 1) * CH[1]],
                            h1v[:, r0 + kh:r0 + kh + ROWS_PER_TILE, 0:W],
                            start=(kh == 0), stop=False)
                    for kh in range(3):
                        nc.tensor.matmul(
                            ps, w2ss[:, kh * CH[1]:(kh + 1) * CH[1]],
                            h1v[0:CH[0], r0 + kh:r0 + kh + ROWS_PER_TILE,
                                2:2 + W],
                            start=False, stop=(kh == 2))
                    nc.scalar.activation(
                        h2v[:, 1 + r0:1 + r0 + ROWS_PER_TILE, 1:1 + W],
                        ps, AF.Lrelu, bias=b2s[:, 0:1], scale=1.0, alpha=ALPHA)

                # 4) layers 3-4: shift-GEMM, 9 taps accumulated in PSUM
                for li, (src, dstv, wsb, bsb, cout) in enumerate((
                    (h2v, h3v, w3s, b3s, CH[2]),
                    (h3v, None, w4s, b4s, CH[3]),
                )):
                    for j in range(NTILE):
                        r0 = j * ROWS_PER_TILE
                        ps = pp.tile([cout, TILEPIX], F32,
                                     name=f"ps{li + 3}_{img}_{j}", tag="ps")
                        for tap, (kh, kw) in enumerate(taps):
                            nc.tensor.matmul(
                                ps, wsb[:, tap * cout:(tap + 1) * cout],
                                src[:, r0 + kh:r0 + kh + ROWS_PER_TILE,
                                    kw:kw + W],
                                start=(tap == 0), stop=(tap == 8))
                        if dstv is not None:
                            nc.scalar.activation(
                                dstv[:, 1 + r0:1 + r0 + ROWS_PER_TILE, 1:1 + W],
                                ps, AF.Lrelu, bias=bsb[:, 0:1], scale=1.0,
                                alpha=ALPHA)
                        else:
                            nc.scalar.activation(
                                h4[:, j * TILEPIX:(j + 1) * TILEPIX], ps,
                                AF.Lrelu, bias=bsb[:, 0:1], scale=1.0,
                                alpha=ALPHA)

                # 4) positional encoding add + store
                nc.vector.tensor_scalar_add(h4, h4, pes[:, t:t + 1])
                nc.sync.dma_start(out=outd[img], in_=h4)

    nc.compile()
    return nc


def _pe_table():
    d = np.arange(CH[3])
    d_even = (d // 2) * 2
    tt = np.arange(T, dtype=np.float64)
    arg = tt[:, None] / np.power(10000.0, d_even / CH[3])
    pe = np.where(d % 2 == 0, np.sin(arg), np.cos(arg))  # [T, D]
    return np.ascontiguousarray(pe.T.astype(np.float32))  # [D, T]


def _prep_consts(w0, b0, w1, b1, w2, b2, w3, b3):
    consts = {
        "w1": np.ascontiguousarray(
            w0.transpose(2, 3, 1, 0).reshape(CIN * 9, CH[0])),
        "w2": np.ascontiguousarray(
            w1.transpose(1, 2, 3, 0).reshape(CH[0], 9 * CH[1])),
        "w3": np.ascontiguousarray(
            w2.transpose(1, 2, 3, 0).reshape(CH[1], 9 * CH[2])),
        "w4": np.ascontiguousarray(
            w3.transpose(1, 2, 3, 0).reshape(CH[2], 9 * CH[3])),
        "b1": np.ascontiguousarray(b0.reshape(CH[0], 1)),
        "b2": np.ascontiguousarray(b1.reshape(CH[1], 1)),
        "b3": np.ascontiguousarray(b2.reshape(CH[2], 1)),
        "b4": np.ascontiguousarray(b3.reshape(CH[3], 1)),
        "pe": _pe_table(),
    }
    return {k: v.astype(np.float32) for k, v in consts.items()}


_prog_cache: dict[int, object] = {}


def _get_program(nimg: int):
    if nimg not in _prog_cache:
        _prog_cache[nimg] = _build(nimg)
    return _prog_cache[nimg]


def make_in_maps(x, w0, b0, w1, b1, w2, b2, w3, b3):
    """Shard the full inputs into the 8 per-core input maps."""
    consts = _prep_consts(w0, b0, w1, b1, w2, b2, w3, b3)
    bpc = B // N_CORES  # batches per core
    in_maps = []
    for c in range(N_CORES):
        xs = np.ascontiguousarray(
            x[c * bpc:(c + 1) * bpc].reshape(bpc * T, CIN, H, W)
        ).astype(np.float32)
        in_maps.append({"xin": xs, **consts})
    return in_maps


def kernel(x, w0, b0, w1, b1, w2, b2, w3, b3):
    nimg = (B // N_CORES) * T
    nc = _get_program(nimg)
    in_maps = make_in_maps(x, w0, b0, w1, b1, w2, b2, w3, b3)
    res = run_bass_kernel_spmd(nc, in_maps, core_ids=list(range(N_CORES)))
    bpc = B // N_CORES
    outs = [
        res.results[c]["out"].reshape(bpc, T, CH[3], H, W)
        for c in range(N_CORES)
    ]
    return np.concatenate(outs, axis=0).astype(np.float32)
